# revision 67
# baseline (speedup 1.0000x reference)
"""Bass/Tile TRN2 kernel for nn_SSEGCNBertClassifier (gnn_message_passing).

Data-parallel over batch: B=32 -> 8 cores x 4 batches. All params replicated.

Math notes (vs reference):
  - layernorm scale/shift folded on host into the Wxx matmul
    (WaW = ln_a*Wxx_w, v = ln_b@Wxx_w + Wxx_b)
  - torch-style unbiased std: rstd via linear seed + 1 Newton step on DVE;
    eps=1e-6 dropped (relative effect ~1e-6).
  - src_mask folded into short_mask on host: short' = short + (src-1)*1e9,
    so masked columns exp to 0 with no separate mask term on device.
  - softmax without max-subtraction (scores bounded ~|15|); normalization
    (1/rowsum, and the 1/H of mean-head message passing via W_w/H on host)
    folded into the head-reduction scalar_tensor_tensor ops.
  - the per-head additive row tanh(asp.k)+bias enters each head's scores
    matmul as a rank-1 (ones x row) accumulation; rows live at partition
    bases 0/32/64 so they are directly addressable as matmul operands.
  - the [B,L,L,H] edge tensor is never materialized: layer-2 message passing
    only needs the head-sum (see baseline derivation).
  - all transposes are PE transposes into paired psum tiles (one DVE copy
    per [128,256] pair); no DMA transposes.
  - all weights ship in 2 packed DRAM blobs (1 bf16 + 1 f32) = 2 DMAs.
"""

import math

import numpy as np

import concourse.bacc as bacc
import concourse.tile as tile
from concourse import mybir
from concourse.bass_utils import run_bass_kernel_spmd

F32 = mybir.dt.float32
BF16 = mybir.dt.bfloat16
NPBF16 = mybir.dt.np(BF16)
AF = mybir.ActivationFunctionType
OP = mybir.AluOpType

H, DK, ATT, D, L, B = 5, 20, 100, 768, 256, 32
NCORES = 8
BC = B // NCORES  # batches per core

# ---- bf16 blob column layout
_BF_SLOTS = [
    ("WaW", 6 * ATT), ("qaugA", 84), ("qaugB", 52), ("kaugA", 84),
    ("kaugB", 52), ("dense_w", DK), ("Ww", ATT), ("Wb_row", ATT),
    ("w12s", 2), ("clf_w", 3), ("w0pad", ATT), ("am", BC * 2),
]
_BF_OFF = {}
_off = 0
for _n, _w in _BF_SLOTS:
    _BF_OFF[_n] = _off
    _off += _w
NBF = _off

# ---- f32 blob column layout
_F_SLOTS = [
    ("v_col", 1), ("dense_b", 1), ("bm", 1), ("Wb_col", 1), ("wa", H),
    ("clf_b", 1), ("rwn", BC),
]
_F_OFF = {}
_off = 0
for _n, _w in _F_SLOTS:
    _F_OFF[_n] = _off
    _off += _w
NF = _off

F32R = mybir.dt.float32r

_IN_SPECS = [
    ("seq", [BC, L, D], BF16),
    ("seqT", [BC, 128, 6 * L], BF16),
    ("ident", [128, 128], BF16),
    ("blob_bf", [128, NBF], BF16),
    ("blob_f", [128, NF], F32),
    ("blob_s", [128, BC * 2 * L], BF16),
]


# ----------------------------------------------------------------- host prep

def _host_prep(inputs):
    f32 = np.float32
    ln_a = inputs["ln_a"].astype(f32)
    ln_b = inputs["ln_b"].astype(f32)
    Wxx_w = inputs["Wxx_w"].astype(f32)
    Wxx_b = inputs["Wxx_b"].astype(f32)
    q_w, q_b = inputs["q_w"].astype(f32), inputs["q_b"].astype(f32)
    k_w, k_b = inputs["k_w"].astype(f32), inputs["k_b"].astype(f32)
    Wx_w, Wx_b = inputs["Wx_w"].astype(f32), inputs["Wx_b"].astype(f32)
    W_w, W_b = inputs["W_w"].astype(f32), inputs["W_b"].astype(f32)

    sq = 1.0 / math.sqrt(DK)
    # Head-padded projections: head h occupies output cols [32h, 32h+20) of
    # its A/B tile so each head's scores operands sit at partition base
    # 0/32/64 (a PE requirement). Row 100 of the augmented input is ones and
    # picks up the biases.
    qaug = np.concatenate([q_w * sq, q_b[None] * sq], 0).astype(f32)
    kaug = np.concatenate([k_w, k_b[None]], 0).astype(f32)
    qaugA = np.zeros((ATT + 1, 84), f32)
    kaugA = np.zeros((ATT + 1, 84), f32)
    qaugB = np.zeros((ATT + 1, 52), f32)
    kaugB = np.zeros((ATT + 1, 52), f32)
    for h in range(3):
        qaugA[:, 32 * h:32 * h + DK] = qaug[:, DK * h:DK * (h + 1)]
        kaugA[:, 32 * h:32 * h + DK] = kaug[:, DK * h:DK * (h + 1)]
    for j, h in enumerate((3, 4)):
        qaugB[:, 32 * j:32 * j + DK] = qaug[:, DK * h:DK * (h + 1)]
        kaugB[:, 32 * j:32 * j + DK] = kaug[:, DK * h:DK * (h + 1)]

    blob_bf = np.zeros((128, NBF), NPBF16)

    def put_bf(name, arr):
        a = np.asarray(arr, f32)
        p, w = a.shape
        blob_bf[0:p, _BF_OFF[name]:_BF_OFF[name] + w] = a.astype(NPBF16)

    put_bf("WaW", (ln_a[:, None] * Wxx_w).reshape(6, 128, ATT)
           .transpose(1, 0, 2).reshape(128, 6 * ATT))
    put_bf("qaugA", qaugA)
    put_bf("qaugB", qaugB)
    put_bf("kaugA", kaugA)
    put_bf("kaugB", kaugB)
    put_bf("dense_w", inputs["dense_w"].astype(f32))
    put_bf("Ww", W_w / H)                       # 1/H of mean-head msg passing
    put_bf("Wb_row", W_b.reshape(1, ATT))
    put_bf("w12s", np.stack([Wx_w[H:H + ATT].sum(1),
                             Wx_w[H + ATT:].sum(1)], 1))
    put_bf("clf_w", inputs["clf_w"].astype(f32))
    w0 = ln_a @ Wxx_w                       # colsums of WaW (for LN mean fold)
    put_bf("w0pad", w0.reshape(1, ATT))

    blob_f = np.zeros((128, NF), f32)

    def put_f(name, arr):
        a = np.asarray(arr, f32)
        p, w = a.shape
        blob_f[0:p, _F_OFF[name]:_F_OFF[name] + w] = a

    put_f("v_col", (ln_b @ Wxx_w + Wxx_b).reshape(ATT, 1))
    put_f("dense_b", inputs["dense_b"].astype(f32).reshape(DK, 1))
    put_f("bm", np.full((128, 1), float(inputs["bias_m"][0]), f32))
    put_f("Wb_col", W_b.reshape(ATT, 1))
    put_f("wa", np.broadcast_to(Wx_w[:H].sum(1)[None, :], (128, H)))
    put_f("clf_b", inputs["clf_b"].astype(f32).reshape(3, 1))
    cconst = float(Wx_b.sum())
    was = tuple(float(x) for x in Wx_w[:H].sum(1))

    seq = inputs["sequence_output"].astype(f32)
    short = inputs["short_mask"].astype(f32)[:, 0]          # [B,L,L]
    am = inputs["aspect_mask"].astype(f32)                  # [B,L]
    maskterm = (inputs["src_mask"].astype(f32) - 1.0) * 1e9  # [B,L]
    shortm = short + maskterm[:, None, :]                   # fold src mask

    ident = np.eye(128, dtype=f32).astype(NPBF16)

    per_core = []
    for c in range(NCORES):
        s = slice(c * BC, (c + 1) * BC)
        bf = blob_bf.copy()
        bf[:, _BF_OFF["am"]:_BF_OFF["am"] + BC * 2] = (
            am[s].reshape(BC, 2, 128).transpose(2, 0, 1)
            .reshape(128, BC * 2).astype(NPBF16))
        fl = blob_f.copy()
        rwn = 1.0 / am[s].sum(1)  # [BC]
        fl[:, _F_OFF["rwn"]:_F_OFF["rwn"] + BC] = np.broadcast_to(
            rwn[None, :], (128, BC))
        per_core.append({
            "seq": seq[s].astype(NPBF16),
            "seqT": seq[s].reshape(BC, L, 6, 128).transpose(0, 3, 2, 1)
            .reshape(BC, 128, 6 * L).astype(NPBF16),
            "ident": ident,
            "blob_s": shortm[s].reshape(BC, 2, 128, L).transpose(2, 0, 1, 3)
            .reshape(128, BC * 2 * L).astype(NPBF16),
            "blob_bf": bf,
            "blob_f": fl,
        })
    return per_core, cconst, was


# -------------------------------------------------------------- kernel body

def _emit(tc, io, cconst, was, bc):
    nc = tc.nc
    pools = []

    def pool(name, **kw):
        p = tc.alloc_tile_pool(name=name, **kw)
        pools.append(p)
        return p

    singles = pool("singles", bufs=1)
    sbig = pool("sbig", bufs=3)        # per-batch big sbuf tiles
    sp = pool("spp", bufs=7)           # p tiles
    ssm = pool("ssm", bufs=4)          # small sbuf
    # PSUM slots are bank-granular: 8 banks total, one per tag x buf.
    ps_s = pool("ps_s", bufs=2, space="PSUM")    # scores psum (2 banks)
    ps_ab = pool("ps_ab", bufs=2, space="PSUM")  # a1T/btT accum + transpose
    ps_f = pool("ps_f", bufs=1, space="PSUM")    # front: gT/qkA/qkB (1 bank)
    ps_b = pool("ps_b", bufs=1, space="PSUM")    # back: ax1..g3 (1 bank)
    ps_sm = pool("ps_sm", bufs=2, space="PSUM")  # smalls (2 banks)

    # ---- prologue DMAs: seq0, ident, blobs, seq1-3
    seqx = [singles.tile([128, 2, D], BF16, tag=f"x2_{b}", name=f"x2_{b}")
            for b in range(bc)]
    seqT = [singles.tile([128, 6, L], BF16, tag=f"xT_{b}", name=f"xT_{b}")
            for b in range(bc)]
    nc.sync.dma_start(out=seqx[0],
                      in_=io["seq"].ap()[0].rearrange("(c p) d -> p c d",
                                                      p=128))
    nc.scalar.dma_start(out=seqT[0].rearrange("p c l -> p (c l)"),
                        in_=io["seqT"].ap()[0])
    ident = singles.tile([128, 128], BF16, tag="ident", name="ident")
    nc.sync.dma_start(out=ident, in_=io["ident"].ap())
    blob_bf = singles.tile([128, NBF], BF16, tag="blob_bf", name="blob_bf")
    nc.scalar.dma_start(out=blob_bf, in_=io["blob_bf"].ap())
    blob_f = singles.tile([128, NF], F32, tag="blob_f", name="blob_f")
    nc.sync.dma_start(out=blob_f, in_=io["blob_f"].ap())
    blob_s = singles.tile([128, BC * 2 * L], BF16, tag="blob_s",
                          name="blob_s")
    nc.scalar.dma_start(out=blob_s, in_=io["blob_s"].ap())
    for b in range(1, bc):
        e1, e2 = (nc.sync, nc.scalar) if b % 2 == 0 else (nc.scalar, nc.sync)
        e1.dma_start(out=seqx[b],
                     in_=io["seq"].ap()[b].rearrange("(c p) d -> p c d",
                                                     p=128))
        e2.dma_start(out=seqT[b].rearrange("p c l -> p (c l)"),
                     in_=io["seqT"].ap()[b])

    def bfs(name, p0, p1, c0, c1):
        return blob_bf[p0:p1, _BF_OFF[name] + c0:_BF_OFF[name] + c1]

    def fs(name, p0, p1, c0=0, c1=1):
        return blob_f[p0:p1, _F_OFF[name] + c0:_F_OFF[name] + c1]

    WaW = blob_bf[:, _BF_OFF["WaW"]:_BF_OFF["WaW"] + 6 * ATT].rearrange(
        "p (c k) -> p c k", c=6)
    shortv = blob_s.rearrange("p (b c l) -> p b c l", b=BC, c=2)
    qaugA = bfs("qaugA", 0, ATT + 1, 0, 84)
    qaugB = bfs("qaugB", 0, ATT + 1, 0, 52)
    kaugA = bfs("kaugA", 0, ATT + 1, 0, 84)
    kaugB = bfs("kaugB", 0, ATT + 1, 0, 52)
    dense_w = bfs("dense_w", 0, ATT, 0, DK)
    Ww = bfs("Ww", 0, ATT, 0, ATT)
    Wb_row = bfs("Wb_row", 0, 1, 0, ATT)
    w12s = bfs("w12s", 0, ATT, 0, 2)
    clf_w = bfs("clf_w", 0, ATT, 0, 3)

    # ---- device-built constants
    ones_row = singles.tile([1, L], BF16, tag="ones_row", name="ones_row")
    nc.vector.memset(ones_row, 1.0)
    ones_col = singles.tile([128, 1], BF16, tag="ones_col", name="ones_col")
    nc.vector.memset(ones_col, 1.0)
    # ones rows at partition bases 0/32/64 (matmul requires stationary and
    # moving operands to share a base partition; the additive-row rank-1's
    # moving row lives at base 32h)
    ones65 = singles.tile([65, 128], BF16, tag="ones65", name="ones65")
    nc.vector.memset(ones65, 1.0)
    # gTaug / bdiag: 2 rotating buffers each, constant parts set once here
    gTaugs, bdAs, bdBs = [], [], []
    for i in range(2):
        g = singles.tile([128, L], BF16, tag=f"gTaug{i}", name=f"gTaug{i}")
        nc.gpsimd.memset(g[96:128, :], 0.0)
        # ones row (partition 100) for q/k biases; engine ops need 32-aligned
        # partition bases, so write it via SWDGE dma (prologue-only)
        nc.gpsimd.dma_start(out=g[ATT:ATT + 1, :], in_=ones_row)
        gTaugs.append(g)
        a = singles.tile([84, 65], BF16, tag=f"bdA{i}", name=f"bdA{i}")
        nc.gpsimd.memset(a, 0.0)
        bdAs.append(a)
        bl = singles.tile([52, 33], BF16, tag=f"bdB{i}", name=f"bdB{i}")
        nc.gpsimd.memset(bl, 0.0)
        bdBs.append(bl)
    out_all = singles.tile([3, bc], F32, tag="out_all", name="out_all")
    # PE p-state warmup: keep the tensor engine continuously busy from t=0
    # until the first real matmuls (~6-7us) so they run at full clock
    # instead of the cold 0.65GHz p-state
    warm_in = singles.tile([128, 128], BF16, tag="warm_in", name="warm_in")
    nc.vector.memset(warm_in, 0.0)
    warm_ps = ps_b.tile([128, 128], F32, tag="back", name="warm_ps")
    for i in range(110):
        nc.tensor.matmul(warm_ps, warm_in, warm_in, start=True, stop=True)
    st4s, go2Ts = [], []
    for i in range(2):
        s4 = singles.tile([128, 128], BF16, tag=f"st4_{i}", name=f"st4_{i}")
        nc.vector.memset(s4, 0.0)
        st4s.append(s4)
        g2 = singles.tile([128, L], BF16, tag=f"go2T_{i}", name=f"go2T_{i}")
        nc.gpsimd.memset(g2[96:128, :], 0.0)
        go2Ts.append(g2)

    def front(b):
        st = {}
        x2 = seqx[b]
        gTaug = gTaugs[b % 2]

        # for batch 0: gT main matmuls first — they only need xT + WaW
        # (ready early) and must not queue behind the st4 transposes on PE
        xT = seqT[b]
        gT_ps = None
        if b == 0:
            gT_ps = ps_f.tile([ATT, L], F32, tag="front", name="gT_ps")
            for fc in range(6):
                nc.tensor.matmul(gT_ps, WaW[:, fc, :], xT[:, fc, :],
                                 start=(fc == 0), stop=False)

        # ---------------------------------------- layernorm stats + rstd
        # (the LN itself is folded: gT = rstd * (WaW^T @ xT + w0 x (-mean))
        #  + v, with the per-token rstd applied via a broadcast tile)
        st4 = st4s[b % 2]
        mvs = ssm.tile([128, 2, 2], F32, tag="mvs")
        for ic in range(2):
            stats = ssm.tile([128, 2, 6], F32, tag="stats")
            nc.vector.bn_stats(out=stats[:, 0, :], in_=x2[:, ic, 0:512])
            nc.vector.bn_stats(out=stats[:, 1, :], in_=x2[:, ic, 512:D])
            nc.vector.bn_aggr(out=mvs[:, ic, :], in_=stats)
        # rstd = rsqrt(var*n/(n-1)): linear seed + 1 Newton step
        vc = ssm.tile([128, 2], F32, tag="vc")
        nc.vector.tensor_scalar_mul(out=vc, in0=mvs[:, :, 1],
                                    scalar1=float(D) / (D - 1))
        y = ssm.tile([128, 2], F32, tag="y")
        nc.vector.tensor_scalar(out=y, in0=vc, scalar1=-0.5, scalar2=1.5,
                                op0=OP.mult, op1=OP.add)
        y2 = ssm.tile([128, 2], F32, tag="y2")
        nc.vector.tensor_mul(out=y2, in0=y, in1=y)
        nc.vector.tensor_mul(out=y2, in0=y2, in1=vc)
        nc.vector.tensor_scalar(out=y2, in0=y2, scalar1=-0.5, scalar2=1.5,
                                op0=OP.mult, op1=OP.add)
        # rstd -> st4 cols 0/32; -mean -> st4 cols 64/96
        nc.vector.tensor_mul(out=st4[:, 0:33:32], in0=y, in1=y2)
        nc.vector.tensor_scalar_mul(out=st4[:, 64:97:32], in0=mvs[:, :, 0],
                                    scalar1=-1.0)
        # transpose the per-token scalar columns; 4 narrow transposes so
        # each row lands at partition 0 (matmuls with base!=0 operands into
        # column-sliced psum crash the HW)
        rows4 = ssm.tile([1, 4, 128], BF16, tag="rows4")
        for q in range(4):
            qT_ps = ps_sm.tile([32, 128], BF16, tag="small", name=f"qT{q}")
            nc.tensor.transpose(qT_ps, st4[:, 32 * q:32 * (q + 1)], ident)
            nc.vector.tensor_copy(out=rows4[:, q, :], in_=qT_ps[0:1, :])

        # ---------------------------------------- gT matmuls + rank-1
        if gT_ps is None:
            gT_ps = ps_f.tile([ATT, L], F32, tag="front", name="gT_ps")
            for fc in range(6):
                nc.tensor.matmul(gT_ps, WaW[:, fc, :], xT[:, fc, :],
                                 start=(fc == 0), stop=False)
        for ic in range(2):
            nc.tensor.matmul(gT_ps[:, ic * 128:(ic + 1) * 128],
                             bfs("w0pad", 0, 1, 0, ATT),
                             rows4[:, 2 + ic, :],
                             start=False, stop=(ic == 1))
        abc_ps = ps_sm.tile([128, L], F32, tag="small", name="abc_ps")
        for ic in range(2):
            nc.tensor.matmul(abc_ps[:, ic * 128:(ic + 1) * 128],
                             ones_row[:, 0:128], rows4[:, ic, :],
                             start=True, stop=True)
        gt0 = sbig.tile([ATT, L], F32, tag="gt0")
        nc.vector.tensor_copy(out=gt0, in_=gT_ps)
        gt_sb = sbig.tile([ATT, L], BF16, tag="gt_sb")
        nc.vector.tensor_mul(out=gt_sb, in0=gt0, in1=abc_ps[0:ATT, :])
        nc.scalar.activation(out=gTaug[0:ATT, :], in_=gt_sb, func=AF.Identity,
                             bias=fs("v_col", 0, ATT))
        g_nat = sbig.tile([128, 2, 128], BF16, tag="g_nat")
        tpg = ps_f.tile([128, L], BF16, tag="front", name="tpg")
        for ic in range(2):
            nc.tensor.transpose(tpg[:, ic * 128:(ic + 1) * 128],
                                gTaug[:, ic * 128:(ic + 1) * 128], ident)
        nc.vector.tensor_copy(out=g_nat.rearrange("p a b -> p (a b)"),
                              in_=tpg)

        # ---------------------------------------- q/k (pair psums)
        qkA_ps = ps_f.tile([84, 2, L], F32, tag="front", name="qkA_ps")
        nc.tensor.matmul(qkA_ps[:, 0, :], qaugA, gTaug[0:ATT + 1, :],
                         start=True, stop=True)
        nc.tensor.matmul(qkA_ps[:, 1, :], kaugA, gTaug[0:ATT + 1, :],
                         start=True, stop=True)
        qkA = sbig.tile([84, 2, L], BF16, tag="qkA_sb")
        nc.scalar.copy(out=qkA.rearrange("p a b -> p (a b)"),
                       in_=qkA_ps.rearrange("p a b -> p (a b)"))
        qkB_ps = ps_f.tile([52, 2, L], F32, tag="front", name="qkB_ps")
        nc.tensor.matmul(qkB_ps[:, 0, :], qaugB, gTaug[0:ATT + 1, :],
                         start=True, stop=True)
        nc.tensor.matmul(qkB_ps[:, 1, :], kaugB, gTaug[0:ATT + 1, :],
                         start=True, stop=True)
        qkB = sbig.tile([52, 2, L], BF16, tag="qkB_sb")
        nc.scalar.copy(out=qkB.rearrange("p a b -> p (a b)"),
                       in_=qkB_ps.rearrange("p a b -> p (a b)"))

        # ---------------------------------------- aspect path
        asp_ps = ps_sm.tile([ATT, 1], F32, tag="small", name="asp_ps")
        for ic in range(2):
            nc.tensor.matmul(asp_ps, g_nat[:, ic, 0:ATT],
                             bfs("am", 0, 128, 2 * b + ic, 2 * b + ic + 1),
                             start=(ic == 0), stop=(ic == 1))
        aspect_sb = ssm.tile([ATT, 1], BF16, tag="aspect_sb")
        nc.vector.tensor_scalar_mul(out=aspect_sb, in0=asp_ps,
                                    scalar1=fs("rwn", 0, ATT, b, b + 1))
        asp2_ps = ps_sm.tile([DK, 1], F32, tag="small", name="asp2_ps")
        nc.tensor.matmul(asp2_ps, dense_w, aspect_sb, start=True, stop=True)
        asp_sb = ssm.tile([DK, 1], BF16, tag="asp_sb")
        nc.vector.tensor_scalar_add(out=asp_sb, in0=asp2_ps,
                                    scalar1=fs("dense_b", 0, DK))
        bdA, bdB = bdAs[b % 2], bdBs[b % 2]
        for h in range(3):
            nc.gpsimd.tensor_copy(
                out=bdA[32 * h:32 * h + DK, 32 * h:32 * h + 1], in_=asp_sb)
        for j in range(2):
            nc.gpsimd.tensor_copy(
                out=bdB[32 * j:32 * j + DK, 32 * j:32 * j + 1], in_=asp_sb)
        kd_ps = ps_sm.tile([65, 2, L], F32, tag="small", name="kd_ps")
        nc.tensor.matmul(kd_ps[:, 0, :], bdA, qkA[0:84, 1, :],
                         start=True, stop=True)
        nc.tensor.matmul(kd_ps[0:33, 1, :], bdB, qkB[0:52, 1, :],
                         start=True, stop=True)
        rows2 = ssm.tile([65, 2, L], BF16, tag="rows2")
        nc.scalar.activation(out=rows2.rearrange("p a b -> p (a b)"),
                             in_=kd_ps.rearrange("p a b -> p (a b)"),
                             func=AF.Tanh, bias=fs("bm", 0, 65))
        rowsA = rows2[:, 0, :]
        rowsB = rows2[0:33, 1, :]

        st["g_nat"] = g_nat
        st["qkA"] = qkA
        st["qkB"] = qkB
        st["rowsA"] = rowsA
        st["rowsB"] = rowsB
        return st

    def back_scores(st, b):
        g_nat = st["g_nat"]
        qkA, qkB = st["qkA"], st["qkB"]
        rowsA, rowsB = st["rowsA"], st["rowsB"]

        # ---------------------------------------- scores/softmax + reduce
        # p_h = exp(short' + qk + row_h). The softmax normalization AND the
        # transpose AND the head reduction all happen in one PE pass:
        #   a1T[j,i] = sum_h (p_h^T @ diag(rrs_h))[j,i]
        #   btT[j,i] = sum_h (p_h^T @ diag(wa_h*rrs_h))[j,i]
        # with diag tiles built as ident * scalar-column on DVE.
        rss, pss = [], []
        for ic in range(2):
            rs = ssm.tile([128, H], F32, tag="rs", bufs=5)
            ps = []
            for h in range(H):
                s_ps = ps_s.tile([128, L], F32, tag="s_ps")
                nc.tensor.matmul(s_ps, ident, shortv[:, b, ic, :],
                                 start=True, stop=False)
                if h < 3:
                    j = 32 * h
                    rowh = rowsA[j:j + 1, :]
                    qh = qkA[j:j + DK, 0, ic * 128:(ic + 1) * 128]
                    kh = qkA[j:j + DK, 1, :]
                else:
                    j = 32 * (h - 3)
                    rowh = rowsB[j:j + 1, :]
                    qh = qkB[j:j + DK, 0, ic * 128:(ic + 1) * 128]
                    kh = qkB[j:j + DK, 1, :]
                nc.tensor.matmul(s_ps, ones65[j:j + 1, :], rowh,
                                 start=False, stop=False)
                nc.tensor.matmul(s_ps, qh, kh, start=False, stop=True)
                p = sp.tile([128, L], BF16, tag="p", bufs=22)
                nc.scalar.activation(out=p, in_=s_ps, func=AF.Exp,
                                     accum_out=rs[:, h:h + 1])
                ps.append(p)
            rss.append(rs)
            pss.append(ps)
        st["rss"] = rss
        st["pss"] = pss
        return st

    def back_scores_reduce(st, b):
        abls = [ps_ab.tile([128, 2, 2, 128], F32, tag="abT", name=f"abT{jc}")
                for jc in range(2)]
        for ic in range(2):
            rs = st["rss"][ic]
            ps = st["pss"][ic]
            rrs = ssm.tile([128, H], F32, tag="rrs")
            nc.vector.reciprocal(out=rrs, in_=rs)
            # one psum accumulation group may be open per bank at a time:
            # run the a1 groups (both jc banks) to completion, then bt
            Rs = []
            for h in range(H):
                R = sp.tile([128, 128], BF16, tag="R", bufs=12)
                nc.vector.tensor_scalar_mul(out=R, in0=ident,
                                            scalar1=rrs[:, h:h + 1])
                Rs.append(R)
            for h in range(H):
                for jc in range(2):
                    pj = ps[h][:, jc * 128:(jc + 1) * 128]
                    nc.tensor.matmul(abls[jc][:, 0, ic, :], pj, Rs[h],
                                     start=(h == 0), stop=(h == 4))
            Rws = []
            for h in range(H):
                Rw = sp.tile([128, 128], BF16, tag="R", bufs=12)
                nc.vector.tensor_scalar_mul(out=Rw, in0=Rs[h], scalar1=was[h])
                Rws.append(Rw)
            for h in range(H):
                for jc in range(2):
                    pj = ps[h][:, jc * 128:(jc + 1) * 128]
                    nc.tensor.matmul(abls[jc][:, 1, ic, :], pj, Rws[h],
                                     start=(h == 0), stop=(h == 4))
        st["abls"] = abls

    def back_reduce(st, b, last=False):
        back_scores_reduce(st, b)
        abls = st["abls"]
        g_nat = st["g_nat"]
        ab_sb = []
        for jc in range(2):
            t = sbig.tile([128, 2, 2, 128], BF16, tag=f"ab{jc}",
                          name=f"ab{jc}")
            if last and jc == 1:
                nc.scalar.copy(out=t.rearrange("p a b c -> p (a b c)"),
                               in_=abls[jc].rearrange("p a b c -> p (a b c)"))
            else:
                nc.vector.tensor_copy(
                    out=t.rearrange("p a b c -> p (a b c)"),
                    in_=abls[jc].rearrange("p a b c -> p (a b c)"))
            ab_sb.append(t)
        a1T = [ab_sb[jc][:, 0] for jc in range(2)]
        btT = [ab_sb[jc][:, 1] for jc in range(2)]

        # ---------------------------------------- Ax1T (1/H in Ww)
        ax1_ps = ps_b.tile([ATT, L], F32, tag="back")
        for jc in range(2):
            nc.tensor.matmul(ax1_ps, g_nat[:, jc, 0:ATT], a1T[jc],
                             start=(jc == 0), stop=(jc == 1))
        ax1_sb = sbig.tile([ATT, L], BF16, tag="ax1_sb")
        if last:
            nc.scalar.copy(out=ax1_sb, in_=ax1_ps)
        else:
            nc.vector.tensor_copy(out=ax1_sb, in_=ax1_ps)

        # ---------------------------------------- go2 (both layouts)
        go2T_ps = ps_b.tile([ATT, L], F32, tag="back")
        nc.tensor.matmul(go2T_ps, Ww, ax1_sb, start=True, stop=True)
        go2T = go2Ts[b % 2]
        if last:
            nc.scalar.activation(out=go2T[0:ATT, :], in_=go2T_ps,
                                 func=AF.Relu, bias=fs("Wb_col", 0, ATT))
        else:
            nc.vector.tensor_scalar(out=go2T[0:ATT, :], in0=go2T_ps,
                                    scalar1=fs("Wb_col", 0, ATT), scalar2=0.0,
                                    op0=OP.add, op1=OP.max)
        go2n = sbig.tile([128, L], BF16, tag="go2n")
        tpn = ps_ab.tile([128, L], BF16, tag="abT", name="tpn")
        for ic in range(2):
            nc.tensor.transpose(tpn[:, ic * 128:(ic + 1) * 128],
                                go2T[:, ic * 128:(ic + 1) * 128], ident)
        nc.vector.tensor_copy(out=go2n, in_=tpn)

        # ---------------------------------------- layer-2 rank-1 terms
        s2r_ps = ps_sm.tile([1, L], F32, tag="small", name="s2r_ps")
        nc.tensor.matmul(s2r_ps, w12s[:, 1:2], go2T[0:ATT, :], start=True,
                         stop=True)
        s2c_row = ssm.tile([1, L], BF16, tag="s2c_row")
        nc.vector.tensor_scalar_add(out=s2c_row, in0=s2r_ps, scalar1=cconst)
        s1c = []
        for jc in range(2):
            sc_ps = ps_sm.tile([128, 2], F32, tag="small", name="s1c_ps")
            nc.tensor.matmul(sc_ps, go2T[0:ATT, jc * 128:(jc + 1) * 128],
                             w12s, start=True, stop=True)
            t = ssm.tile([128, 1], BF16, tag=f"s1c{jc}", name=f"s1c{jc}")
            nc.vector.tensor_copy(out=t, in_=sc_ps[:, 0:1])
            s1c.append(t)
        tr_ps = ps_sm.tile([1, ATT], F32, tag="small", name="tr1_ps")
        for jc in range(2):
            nc.tensor.matmul(tr_ps, s1c[jc],
                             go2n[:, jc * 128:jc * 128 + ATT],
                             start=(jc == 0), stop=(jc == 1))
        cs_ps = ps_sm.tile([1, ATT], F32, tag="small", name="cs1_ps")
        for jc in range(2):
            nc.tensor.matmul(cs_ps, ones_col,
                             go2n[:, jc * 128:jc * 128 + ATT],
                             start=(jc == 0), stop=(jc == 1))
        tr_sb = ssm.tile([1, ATT], BF16, tag="tr_sb")
        nc.vector.tensor_copy(out=tr_sb, in_=tr_ps)
        cs_sb = ssm.tile([1, ATT], BF16, tag="cs_sb")
        nc.vector.tensor_copy(out=cs_sb, in_=cs_ps)

        # ---------------------------------------- Ax2T
        ax2_ps = ps_b.tile([ATT, L], F32, tag="back")
        for jc in range(2):
            nc.tensor.matmul(ax2_ps, go2n[:, jc * 128:jc * 128 + ATT],
                             btT[jc], start=(jc == 0), stop=False)
        nc.tensor.matmul(ax2_ps, tr_sb, ones_row, start=False, stop=False)
        nc.tensor.matmul(ax2_ps, cs_sb, s2c_row, start=False, stop=True)
        ax2_sb = sbig.tile([ATT, L], BF16, tag="ax2_sb")
        if last:
            nc.scalar.copy(out=ax2_sb, in_=ax2_ps)
        else:
            nc.vector.tensor_copy(out=ax2_sb, in_=ax2_ps)

        # ---------------------------------------- go3 + readout
        g3s = []
        for ic in range(2):
            g3_ps = ps_b.tile([128, ATT], F32, tag="back", name=f"g3_{ic}")
            nc.tensor.matmul(g3_ps, ax2_sb[:, ic * 128:(ic + 1) * 128],
                             Ww, start=True, stop=False)
            nc.tensor.matmul(g3_ps, ones_row[:, 0:128], Wb_row,
                             start=False, stop=True)
            g3 = sp.tile([128, ATT], BF16, tag="g3")
            if last:
                nc.scalar.activation(out=g3, in_=g3_ps, func=AF.Relu)
            else:
                nc.vector.tensor_scalar(out=g3, in0=g3_ps, scalar1=0.0,
                                        scalar2=0.0, op0=OP.max, op1=OP.add)
            g3s.append(g3)
        out1_ps = ps_sm.tile([ATT, 1], F32, tag="small", name="out1_ps")
        for ic in range(2):
            nc.tensor.matmul(out1_ps, g3s[ic],
                             bfs("am", 0, 128, 2 * b + ic, 2 * b + ic + 1),
                             start=(ic == 0), stop=(ic == 1))
        out1_sb = ssm.tile([ATT, 1], BF16, tag="out1_sb")
        nc.vector.tensor_copy(out=out1_sb, in_=out1_ps)
        clf_ps = ps_sm.tile([3, 1], F32, tag="small", name="clf_ps")
        nc.tensor.matmul(clf_ps, clf_w, out1_sb, start=True, stop=True)
        nc.vector.scalar_tensor_tensor(
            out=out_all[:, b:b + 1], in0=clf_ps, scalar=fs("rwn", 0, 3, b, b + 1),
            in1=fs("clf_b", 0, 3), op0=OP.mult, op1=OP.add)

    sts = [front(0), front(1)]
    back_scores(sts[0], 0)
    for b in range(bc):
        if b + 2 < bc:
            sts.append(front(b + 2))
        if b + 1 < bc:
            back_scores(sts[b + 1], b + 1)
        back_reduce(sts[b], b, last=(b == bc - 1))

    nc.gpsimd.dma_start(out=io["out"].ap().rearrange("b k -> k b"),
                        in_=out_all)

    for p in reversed(pools):
        p.release()


# ------------------------------------------------------------------- driver

_CACHE = {}


def build(cconst, was, bc=BC, num_devices=NCORES, debug=False):
    key = (round(cconst, 12), was, bc, num_devices)
    if key in _CACHE:
        return _CACHE[key]
    nc = bacc.Bacc("TRN2", target_bir_lowering=False, debug=debug,
                   num_devices=num_devices)
    io = {}
    for name, shape, dt in _IN_SPECS:
        io[name] = nc.dram_tensor(name, list(shape), dt, kind="ExternalInput")
    io["out"] = nc.dram_tensor("out", [bc, 3], F32, kind="ExternalOutput")
    with tile.TileContext(nc) as tc:
        _emit(tc, io, cconst, was, bc)
    nc.compile()
    _CACHE[key] = (nc, io)
    return nc, io


def run(inputs, **kwargs):
    per_core, cconst, was = _host_prep(inputs)
    nc, _ = build(cconst, was)
    res = run_bass_kernel_spmd(nc, per_core, core_ids=list(range(NCORES)),
                               **kwargs)
    return np.concatenate([r["out"] for r in res.results], axis=0), res


def kernel(**inputs):
    return run(inputs)[0]


# revision 68
# speedup vs baseline: 1.0320x; 1.0320x over previous
"""Bass/Tile TRN2 kernel for nn_SSEGCNBertClassifier (gnn_message_passing).

Data-parallel over batch: B=32 -> 8 cores x 4 batches. All params replicated.

Math notes (vs reference):
  - layernorm scale/shift folded on host into the Wxx matmul
    (WaW = ln_a*Wxx_w, v = ln_b@Wxx_w + Wxx_b)
  - torch-style unbiased std: rstd via linear seed + 1 Newton step on DVE;
    eps=1e-6 dropped (relative effect ~1e-6).
  - src_mask folded into short_mask on host: short' = short + (src-1)*1e9,
    so masked columns exp to 0 with no separate mask term on device.
  - softmax without max-subtraction (scores bounded ~|15|); normalization
    (1/rowsum, and the 1/H of mean-head message passing via W_w/H on host)
    folded into the head-reduction scalar_tensor_tensor ops.
  - the per-head additive row tanh(asp.k)+bias enters each head's scores
    matmul as a rank-1 (ones x row) accumulation; rows live at partition
    bases 0/32/64 so they are directly addressable as matmul operands.
  - the [B,L,L,H] edge tensor is never materialized: layer-2 message passing
    only needs the head-sum (see baseline derivation).
  - all transposes are PE transposes into paired psum tiles (one DVE copy
    per [128,256] pair); no DMA transposes.
  - all weights ship in 2 packed DRAM blobs (1 bf16 + 1 f32) = 2 DMAs.
"""

import math

import numpy as np

import concourse.bacc as bacc
import concourse.tile as tile
from concourse import mybir
from concourse.bass_utils import run_bass_kernel_spmd

F32 = mybir.dt.float32
BF16 = mybir.dt.bfloat16
NPBF16 = mybir.dt.np(BF16)
AF = mybir.ActivationFunctionType
OP = mybir.AluOpType

H, DK, ATT, D, L, B = 5, 20, 100, 768, 256, 32
NCORES = 8
BC = B // NCORES  # batches per core

# ---- bf16 blob column layout
_BF_SLOTS = [
    ("WaW", 6 * ATT), ("qaugA", 84), ("qaugB", 52), ("kaugA", 84),
    ("kaugB", 52), ("dense_w", DK), ("Ww", ATT), ("Wb_row", ATT),
    ("w12s", 2), ("clf_w", 3), ("w0pad", ATT), ("am", BC * 2),
]
_BF_OFF = {}
_off = 0
for _n, _w in _BF_SLOTS:
    _BF_OFF[_n] = _off
    _off += _w
NBF = _off

# ---- f32 blob column layout
_F_SLOTS = [
    ("v_col", 1), ("dense_b", 1), ("bm", 1), ("Wb_col", 1), ("wa", H),
    ("clf_b", 1), ("rwn", BC),
]
_F_OFF = {}
_off = 0
for _n, _w in _F_SLOTS:
    _F_OFF[_n] = _off
    _off += _w
NF = _off

F32R = mybir.dt.float32r

_IN_SPECS = [
    ("seq", [BC, L, D], BF16),
    ("seqT", [BC, 128, 6 * L], BF16),
    ("ident", [128, 128], BF16),
    ("blob_bf", [128, NBF], BF16),
    ("blob_f", [128, NF], F32),
    ("blob_s", [128, BC * 2 * L], BF16),
]


# ----------------------------------------------------------------- host prep

def _host_prep(inputs):
    f32 = np.float32
    ln_a = inputs["ln_a"].astype(f32)
    ln_b = inputs["ln_b"].astype(f32)
    Wxx_w = inputs["Wxx_w"].astype(f32)
    Wxx_b = inputs["Wxx_b"].astype(f32)
    q_w, q_b = inputs["q_w"].astype(f32), inputs["q_b"].astype(f32)
    k_w, k_b = inputs["k_w"].astype(f32), inputs["k_b"].astype(f32)
    Wx_w, Wx_b = inputs["Wx_w"].astype(f32), inputs["Wx_b"].astype(f32)
    W_w, W_b = inputs["W_w"].astype(f32), inputs["W_b"].astype(f32)

    sq = 1.0 / math.sqrt(DK)
    # Head-padded projections: head h occupies output cols [32h, 32h+20) of
    # its A/B tile so each head's scores operands sit at partition base
    # 0/32/64 (a PE requirement). Row 100 of the augmented input is ones and
    # picks up the biases.
    qaug = np.concatenate([q_w * sq, q_b[None] * sq], 0).astype(f32)
    kaug = np.concatenate([k_w, k_b[None]], 0).astype(f32)
    qaugA = np.zeros((ATT + 1, 84), f32)
    kaugA = np.zeros((ATT + 1, 84), f32)
    qaugB = np.zeros((ATT + 1, 52), f32)
    kaugB = np.zeros((ATT + 1, 52), f32)
    for h in range(3):
        qaugA[:, 32 * h:32 * h + DK] = qaug[:, DK * h:DK * (h + 1)]
        kaugA[:, 32 * h:32 * h + DK] = kaug[:, DK * h:DK * (h + 1)]
    for j, h in enumerate((3, 4)):
        qaugB[:, 32 * j:32 * j + DK] = qaug[:, DK * h:DK * (h + 1)]
        kaugB[:, 32 * j:32 * j + DK] = kaug[:, DK * h:DK * (h + 1)]

    blob_bf = np.zeros((128, NBF), NPBF16)

    def put_bf(name, arr):
        a = np.asarray(arr, f32)
        p, w = a.shape
        blob_bf[0:p, _BF_OFF[name]:_BF_OFF[name] + w] = a.astype(NPBF16)

    put_bf("WaW", (ln_a[:, None] * Wxx_w).reshape(6, 128, ATT)
           .transpose(1, 0, 2).reshape(128, 6 * ATT))
    put_bf("qaugA", qaugA)
    put_bf("qaugB", qaugB)
    put_bf("kaugA", kaugA)
    put_bf("kaugB", kaugB)
    put_bf("dense_w", inputs["dense_w"].astype(f32))
    put_bf("Ww", W_w / H)                       # 1/H of mean-head msg passing
    put_bf("Wb_row", W_b.reshape(1, ATT))
    put_bf("w12s", np.stack([Wx_w[H:H + ATT].sum(1),
                             Wx_w[H + ATT:].sum(1)], 1))
    put_bf("clf_w", inputs["clf_w"].astype(f32))
    w0 = ln_a @ Wxx_w                       # colsums of WaW (for LN mean fold)
    put_bf("w0pad", w0.reshape(1, ATT))

    blob_f = np.zeros((128, NF), f32)

    def put_f(name, arr):
        a = np.asarray(arr, f32)
        p, w = a.shape
        blob_f[0:p, _F_OFF[name]:_F_OFF[name] + w] = a

    put_f("v_col", (ln_b @ Wxx_w + Wxx_b).reshape(ATT, 1))
    put_f("dense_b", inputs["dense_b"].astype(f32).reshape(DK, 1))
    put_f("bm", np.full((128, 1), float(inputs["bias_m"][0]), f32))
    put_f("Wb_col", W_b.reshape(ATT, 1))
    put_f("wa", np.broadcast_to(Wx_w[:H].sum(1)[None, :], (128, H)))
    put_f("clf_b", inputs["clf_b"].astype(f32).reshape(3, 1))
    cconst = float(Wx_b.sum())
    was = tuple(float(x) for x in Wx_w[:H].sum(1))

    seq = inputs["sequence_output"].astype(f32)
    short = inputs["short_mask"].astype(f32)[:, 0]          # [B,L,L]
    am = inputs["aspect_mask"].astype(f32)                  # [B,L]
    maskterm = (inputs["src_mask"].astype(f32) - 1.0) * 1e9  # [B,L]
    shortm = short + maskterm[:, None, :]                   # fold src mask

    ident = np.eye(128, dtype=f32).astype(NPBF16)

    per_core = []
    for c in range(NCORES):
        s = slice(c * BC, (c + 1) * BC)
        bf = blob_bf.copy()
        bf[:, _BF_OFF["am"]:_BF_OFF["am"] + BC * 2] = (
            am[s].reshape(BC, 2, 128).transpose(2, 0, 1)
            .reshape(128, BC * 2).astype(NPBF16))
        fl = blob_f.copy()
        rwn = 1.0 / am[s].sum(1)  # [BC]
        fl[:, _F_OFF["rwn"]:_F_OFF["rwn"] + BC] = np.broadcast_to(
            rwn[None, :], (128, BC))
        per_core.append({
            "seq": seq[s].astype(NPBF16),
            "seqT": seq[s].reshape(BC, L, 6, 128).transpose(0, 3, 2, 1)
            .reshape(BC, 128, 6 * L).astype(NPBF16),
            "ident": ident,
            "blob_s": shortm[s].reshape(BC, 2, 128, L).transpose(2, 0, 1, 3)
            .reshape(128, BC * 2 * L).astype(NPBF16),
            "blob_bf": bf,
            "blob_f": fl,
        })
    return per_core, cconst, was


# -------------------------------------------------------------- kernel body

def _emit(tc, io, cconst, was, bc):
    nc = tc.nc
    pools = []

    def pool(name, **kw):
        p = tc.alloc_tile_pool(name=name, **kw)
        pools.append(p)
        return p

    singles = pool("singles", bufs=1)
    sbig = pool("sbig", bufs=3)        # per-batch big sbuf tiles
    sp = pool("spp", bufs=7)           # p tiles
    ssm = pool("ssm", bufs=4)          # small sbuf
    # PSUM slots are bank-granular: 8 banks total, one per tag x buf.
    ps_s = pool("ps_s", bufs=2, space="PSUM")    # scores psum (2 banks)
    ps_ab = pool("ps_ab", bufs=2, space="PSUM")  # a1T/btT accum + transpose
    ps_f = pool("ps_f", bufs=1, space="PSUM")    # front: gT/qkA/qkB (1 bank)
    ps_b = pool("ps_b", bufs=1, space="PSUM")    # back: ax1..g3 (1 bank)
    ps_sm = pool("ps_sm", bufs=2, space="PSUM")  # smalls (2 banks)

    # ---- prologue DMAs: seq0, ident, blobs, seq1-3
    seqx = [singles.tile([128, 2, D], BF16, tag=f"x2_{b}", name=f"x2_{b}")
            for b in range(bc)]
    seqT = [singles.tile([128, 6, L], BF16, tag=f"xT_{b}", name=f"xT_{b}")
            for b in range(bc)]
    nc.sync.dma_start(out=seqx[0],
                      in_=io["seq"].ap()[0].rearrange("(c p) d -> p c d",
                                                      p=128))
    nc.scalar.dma_start(out=seqT[0].rearrange("p c l -> p (c l)"),
                        in_=io["seqT"].ap()[0])
    ident = singles.tile([128, 128], BF16, tag="ident", name="ident")
    nc.sync.dma_start(out=ident, in_=io["ident"].ap())
    blob_bf = singles.tile([128, NBF], BF16, tag="blob_bf", name="blob_bf")
    nc.scalar.dma_start(out=blob_bf, in_=io["blob_bf"].ap())
    blob_f = singles.tile([128, NF], F32, tag="blob_f", name="blob_f")
    nc.sync.dma_start(out=blob_f, in_=io["blob_f"].ap())
    blob_s = singles.tile([128, BC * 2 * L], BF16, tag="blob_s",
                          name="blob_s")
    nc.scalar.dma_start(out=blob_s, in_=io["blob_s"].ap())
    for b in range(1, bc):
        e1, e2 = (nc.sync, nc.scalar) if b % 2 == 0 else (nc.scalar, nc.sync)
        e1.dma_start(out=seqx[b],
                     in_=io["seq"].ap()[b].rearrange("(c p) d -> p c d",
                                                     p=128))
        e2.dma_start(out=seqT[b].rearrange("p c l -> p (c l)"),
                     in_=io["seqT"].ap()[b])

    def bfs(name, p0, p1, c0, c1):
        return blob_bf[p0:p1, _BF_OFF[name] + c0:_BF_OFF[name] + c1]

    def fs(name, p0, p1, c0=0, c1=1):
        return blob_f[p0:p1, _F_OFF[name] + c0:_F_OFF[name] + c1]

    WaW = blob_bf[:, _BF_OFF["WaW"]:_BF_OFF["WaW"] + 6 * ATT].rearrange(
        "p (c k) -> p c k", c=6)
    shortv = blob_s.rearrange("p (b c l) -> p b c l", b=BC, c=2)
    qaugA = bfs("qaugA", 0, ATT + 1, 0, 84)
    qaugB = bfs("qaugB", 0, ATT + 1, 0, 52)
    kaugA = bfs("kaugA", 0, ATT + 1, 0, 84)
    kaugB = bfs("kaugB", 0, ATT + 1, 0, 52)
    dense_w = bfs("dense_w", 0, ATT, 0, DK)
    Ww = bfs("Ww", 0, ATT, 0, ATT)
    Wb_row = bfs("Wb_row", 0, 1, 0, ATT)
    w12s = bfs("w12s", 0, ATT, 0, 2)
    clf_w = bfs("clf_w", 0, ATT, 0, 3)

    # ---- device-built constants
    ones_row = singles.tile([1, L], BF16, tag="ones_row", name="ones_row")
    nc.vector.memset(ones_row, 1.0)
    ones_col = singles.tile([128, 1], BF16, tag="ones_col", name="ones_col")
    nc.vector.memset(ones_col, 1.0)
    # ones rows at partition bases 0/32/64 (matmul requires stationary and
    # moving operands to share a base partition; the additive-row rank-1's
    # moving row lives at base 32h)
    ones65 = singles.tile([65, 128], BF16, tag="ones65", name="ones65")
    nc.vector.memset(ones65, 1.0)
    # gTaug / bdiag: 2 rotating buffers each, constant parts set once here
    gTaugs, bdAs, bdBs = [], [], []
    for i in range(2):
        g = singles.tile([128, L], BF16, tag=f"gTaug{i}", name=f"gTaug{i}")
        nc.gpsimd.memset(g[96:128, :], 0.0)
        # ones row (partition 100) for q/k biases; engine ops need 32-aligned
        # partition bases, so write it via SWDGE dma (prologue-only)
        nc.gpsimd.dma_start(out=g[ATT:ATT + 1, :], in_=ones_row)
        gTaugs.append(g)
        a = singles.tile([84, 65], BF16, tag=f"bdA{i}", name=f"bdA{i}")
        nc.gpsimd.memset(a, 0.0)
        bdAs.append(a)
        bl = singles.tile([52, 33], BF16, tag=f"bdB{i}", name=f"bdB{i}")
        nc.gpsimd.memset(bl, 0.0)
        bdBs.append(bl)
    out_all = singles.tile([3, bc], F32, tag="out_all", name="out_all")
    st4s, go2Ts = [], []
    for i in range(2):
        s4 = singles.tile([128, 128], BF16, tag=f"st4_{i}", name=f"st4_{i}")
        nc.vector.memset(s4, 0.0)
        st4s.append(s4)
        g2 = singles.tile([128, L], BF16, tag=f"go2T_{i}", name=f"go2T_{i}")
        nc.gpsimd.memset(g2[96:128, :], 0.0)
        go2Ts.append(g2)

    def front(b):
        st = {}
        x2 = seqx[b]
        gTaug = gTaugs[b % 2]

        # for batch 0: gT main matmuls first — they only need xT + WaW
        # (ready early) and must not queue behind the st4 transposes on PE
        xT = seqT[b]
        gT_ps = None
        if b == 0:
            gT_ps = ps_f.tile([ATT, L], F32, tag="front", name="gT_ps")
            for fc in range(6):
                nc.tensor.matmul(gT_ps, WaW[:, fc, :], xT[:, fc, :],
                                 start=(fc == 0), stop=False)

        # ---------------------------------------- layernorm stats + rstd
        # (the LN itself is folded: gT = rstd * (WaW^T @ xT + w0 x (-mean))
        #  + v, with the per-token rstd applied via a broadcast tile)
        st4 = st4s[b % 2]
        mvs = ssm.tile([128, 2, 2], F32, tag="mvs")
        for ic in range(2):
            stats = ssm.tile([128, 2, 6], F32, tag="stats")
            nc.vector.bn_stats(out=stats[:, 0, :], in_=x2[:, ic, 0:512])
            nc.vector.bn_stats(out=stats[:, 1, :], in_=x2[:, ic, 512:D])
            nc.vector.bn_aggr(out=mvs[:, ic, :], in_=stats)
        # rstd = rsqrt(var*n/(n-1)): linear seed + 1 Newton step
        vc = ssm.tile([128, 2], F32, tag="vc")
        nc.vector.tensor_scalar_mul(out=vc, in0=mvs[:, :, 1],
                                    scalar1=float(D) / (D - 1))
        y = ssm.tile([128, 2], F32, tag="y")
        nc.vector.tensor_scalar(out=y, in0=vc, scalar1=-0.5, scalar2=1.5,
                                op0=OP.mult, op1=OP.add)
        y2 = ssm.tile([128, 2], F32, tag="y2")
        nc.vector.tensor_mul(out=y2, in0=y, in1=y)
        nc.vector.tensor_mul(out=y2, in0=y2, in1=vc)
        nc.vector.tensor_scalar(out=y2, in0=y2, scalar1=-0.5, scalar2=1.5,
                                op0=OP.mult, op1=OP.add)
        # rstd -> st4 cols 0/32; -mean -> st4 cols 64/96
        nc.vector.tensor_mul(out=st4[:, 0:33:32], in0=y, in1=y2)
        nc.vector.tensor_scalar_mul(out=st4[:, 64:97:32], in0=mvs[:, :, 0],
                                    scalar1=-1.0)
        # transpose the per-token scalar columns; 4 narrow transposes so
        # each row lands at partition 0 (matmuls with base!=0 operands into
        # column-sliced psum crash the HW)
        rows4 = ssm.tile([1, 4, 128], BF16, tag="rows4")
        for q in range(4):
            qT_ps = ps_sm.tile([32, 128], BF16, tag="small", name=f"qT{q}")
            nc.tensor.transpose(qT_ps, st4[:, 32 * q:32 * (q + 1)], ident)
            nc.vector.tensor_copy(out=rows4[:, q, :], in_=qT_ps[0:1, :])

        # ---------------------------------------- gT matmuls + rank-1
        if gT_ps is None:
            gT_ps = ps_f.tile([ATT, L], F32, tag="front", name="gT_ps")
            for fc in range(6):
                nc.tensor.matmul(gT_ps, WaW[:, fc, :], xT[:, fc, :],
                                 start=(fc == 0), stop=False)
        for ic in range(2):
            nc.tensor.matmul(gT_ps[:, ic * 128:(ic + 1) * 128],
                             bfs("w0pad", 0, 1, 0, ATT),
                             rows4[:, 2 + ic, :],
                             start=False, stop=(ic == 1))
        abc_ps = ps_sm.tile([128, L], F32, tag="small", name="abc_ps")
        for ic in range(2):
            nc.tensor.matmul(abc_ps[:, ic * 128:(ic + 1) * 128],
                             ones_row[:, 0:128], rows4[:, ic, :],
                             start=True, stop=True)
        gt0 = sbig.tile([ATT, L], F32, tag="gt0")
        nc.vector.tensor_copy(out=gt0, in_=gT_ps)
        gt_sb = sbig.tile([ATT, L], BF16, tag="gt_sb")
        nc.vector.tensor_mul(out=gt_sb, in0=gt0, in1=abc_ps[0:ATT, :])
        nc.scalar.activation(out=gTaug[0:ATT, :], in_=gt_sb, func=AF.Identity,
                             bias=fs("v_col", 0, ATT))
        g_nat = sbig.tile([128, 2, 128], BF16, tag="g_nat")
        tpg = ps_f.tile([128, L], BF16, tag="front", name="tpg")
        for ic in range(2):
            nc.tensor.transpose(tpg[:, ic * 128:(ic + 1) * 128],
                                gTaug[:, ic * 128:(ic + 1) * 128], ident)
        nc.vector.tensor_copy(out=g_nat.rearrange("p a b -> p (a b)"),
                              in_=tpg)

        # ---------------------------------------- q/k (pair psums)
        qkA_ps = ps_f.tile([84, 2, L], F32, tag="front", name="qkA_ps")
        nc.tensor.matmul(qkA_ps[:, 0, :], qaugA, gTaug[0:ATT + 1, :],
                         start=True, stop=True)
        nc.tensor.matmul(qkA_ps[:, 1, :], kaugA, gTaug[0:ATT + 1, :],
                         start=True, stop=True)
        qkA = sbig.tile([84, 2, L], BF16, tag="qkA_sb")
        nc.scalar.copy(out=qkA.rearrange("p a b -> p (a b)"),
                       in_=qkA_ps.rearrange("p a b -> p (a b)"))
        qkB_ps = ps_f.tile([52, 2, L], F32, tag="front", name="qkB_ps")
        nc.tensor.matmul(qkB_ps[:, 0, :], qaugB, gTaug[0:ATT + 1, :],
                         start=True, stop=True)
        nc.tensor.matmul(qkB_ps[:, 1, :], kaugB, gTaug[0:ATT + 1, :],
                         start=True, stop=True)
        qkB = sbig.tile([52, 2, L], BF16, tag="qkB_sb")
        nc.scalar.copy(out=qkB.rearrange("p a b -> p (a b)"),
                       in_=qkB_ps.rearrange("p a b -> p (a b)"))

        # ---------------------------------------- aspect path
        asp_ps = ps_sm.tile([ATT, 1], F32, tag="small", name="asp_ps")
        for ic in range(2):
            nc.tensor.matmul(asp_ps, g_nat[:, ic, 0:ATT],
                             bfs("am", 0, 128, 2 * b + ic, 2 * b + ic + 1),
                             start=(ic == 0), stop=(ic == 1))
        aspect_sb = ssm.tile([ATT, 1], BF16, tag="aspect_sb")
        nc.vector.tensor_scalar_mul(out=aspect_sb, in0=asp_ps,
                                    scalar1=fs("rwn", 0, ATT, b, b + 1))
        asp2_ps = ps_sm.tile([DK, 1], F32, tag="small", name="asp2_ps")
        nc.tensor.matmul(asp2_ps, dense_w, aspect_sb, start=True, stop=True)
        asp_sb = ssm.tile([DK, 1], BF16, tag="asp_sb")
        nc.vector.tensor_scalar_add(out=asp_sb, in0=asp2_ps,
                                    scalar1=fs("dense_b", 0, DK))
        bdA, bdB = bdAs[b % 2], bdBs[b % 2]
        for h in range(3):
            nc.gpsimd.tensor_copy(
                out=bdA[32 * h:32 * h + DK, 32 * h:32 * h + 1], in_=asp_sb)
        for j in range(2):
            nc.gpsimd.tensor_copy(
                out=bdB[32 * j:32 * j + DK, 32 * j:32 * j + 1], in_=asp_sb)
        kd_ps = ps_sm.tile([65, 2, L], F32, tag="small", name="kd_ps")
        nc.tensor.matmul(kd_ps[:, 0, :], bdA, qkA[0:84, 1, :],
                         start=True, stop=True)
        nc.tensor.matmul(kd_ps[0:33, 1, :], bdB, qkB[0:52, 1, :],
                         start=True, stop=True)
        rows2 = ssm.tile([65, 2, L], BF16, tag="rows2")
        nc.scalar.activation(out=rows2.rearrange("p a b -> p (a b)"),
                             in_=kd_ps.rearrange("p a b -> p (a b)"),
                             func=AF.Tanh, bias=fs("bm", 0, 65))
        rowsA = rows2[:, 0, :]
        rowsB = rows2[0:33, 1, :]

        st["g_nat"] = g_nat
        st["qkA"] = qkA
        st["qkB"] = qkB
        st["rowsA"] = rowsA
        st["rowsB"] = rowsB
        return st

    def back_scores(st, b):
        g_nat = st["g_nat"]
        qkA, qkB = st["qkA"], st["qkB"]
        rowsA, rowsB = st["rowsA"], st["rowsB"]

        # ---------------------------------------- scores/softmax + reduce
        # p_h = exp(short' + qk + row_h). The softmax normalization AND the
        # transpose AND the head reduction all happen in one PE pass:
        #   a1T[j,i] = sum_h (p_h^T @ diag(rrs_h))[j,i]
        #   btT[j,i] = sum_h (p_h^T @ diag(wa_h*rrs_h))[j,i]
        # with diag tiles built as ident * scalar-column on DVE.
        rss, pss = [], []
        for ic in range(2):
            rs = ssm.tile([128, H], F32, tag="rs", bufs=5)
            ps = []
            for h in range(H):
                s_ps = ps_s.tile([128, L], F32, tag="s_ps")
                nc.tensor.matmul(s_ps, ident, shortv[:, b, ic, :],
                                 start=True, stop=False)
                if h < 3:
                    j = 32 * h
                    rowh = rowsA[j:j + 1, :]
                    qh = qkA[j:j + DK, 0, ic * 128:(ic + 1) * 128]
                    kh = qkA[j:j + DK, 1, :]
                else:
                    j = 32 * (h - 3)
                    rowh = rowsB[j:j + 1, :]
                    qh = qkB[j:j + DK, 0, ic * 128:(ic + 1) * 128]
                    kh = qkB[j:j + DK, 1, :]
                nc.tensor.matmul(s_ps, ones65[j:j + 1, :], rowh,
                                 start=False, stop=False)
                nc.tensor.matmul(s_ps, qh, kh, start=False, stop=True)
                p = sp.tile([128, L], BF16, tag="p", bufs=22)
                nc.scalar.activation(out=p, in_=s_ps, func=AF.Exp,
                                     accum_out=rs[:, h:h + 1])
                ps.append(p)
            rss.append(rs)
            pss.append(ps)
        st["rss"] = rss
        st["pss"] = pss
        return st

    def back_scores_reduce(st, b):
        abls = [ps_ab.tile([128, 2, 2, 128], F32, tag="abT", name=f"abT{jc}")
                for jc in range(2)]
        for ic in range(2):
            rs = st["rss"][ic]
            ps = st["pss"][ic]
            rrs = ssm.tile([128, H], F32, tag="rrs")
            nc.vector.reciprocal(out=rrs, in_=rs)
            # one psum accumulation group may be open per bank at a time:
            # run the a1 groups (both jc banks) to completion, then bt
            Rs = []
            for h in range(H):
                R = sp.tile([128, 128], BF16, tag="R", bufs=12)
                nc.vector.tensor_scalar_mul(out=R, in0=ident,
                                            scalar1=rrs[:, h:h + 1])
                Rs.append(R)
            for h in range(H):
                for jc in range(2):
                    pj = ps[h][:, jc * 128:(jc + 1) * 128]
                    nc.tensor.matmul(abls[jc][:, 0, ic, :], pj, Rs[h],
                                     start=(h == 0), stop=(h == 4))
            if ic == 1:
                # a1T groups (both ic) are closed: copy out now so ax1 can
                # start while the bt groups are still accumulating
                a1sb = []
                for jc in range(2):
                    t = sbig.tile([128, 2, 128], BF16, tag=f"a1s{jc}",
                                  name=f"a1s{jc}")
                    nc.vector.tensor_copy(
                        out=t.rearrange("p a b -> p (a b)"),
                        in_=abls[jc][:, 0].rearrange("p a b -> p (a b)"))
                    a1sb.append(t)
                st["a1sb"] = a1sb
            Rws = []
            for h in range(H):
                Rw = sp.tile([128, 128], BF16, tag="R", bufs=12)
                nc.vector.tensor_scalar_mul(out=Rw, in0=Rs[h], scalar1=was[h])
                Rws.append(Rw)
            for h in range(H):
                for jc in range(2):
                    pj = ps[h][:, jc * 128:(jc + 1) * 128]
                    nc.tensor.matmul(abls[jc][:, 1, ic, :], pj, Rws[h],
                                     start=(h == 0), stop=(h == 4))
        st["abls"] = abls

    def back_reduce(st, b, last=False):
        back_scores_reduce(st, b)
        abls = st["abls"]
        g_nat = st["g_nat"]
        a1T = st["a1sb"]
        btT = []
        for jc in range(2):
            t = sbig.tile([128, 2, 128], BF16, tag=f"bts{jc}",
                          name=f"bts{jc}")
            if last and jc == 1:
                nc.scalar.copy(out=t.rearrange("p a b -> p (a b)"),
                               in_=abls[jc][:, 1].rearrange("p a b -> p (a b)"))
            else:
                nc.vector.tensor_copy(
                    out=t.rearrange("p a b -> p (a b)"),
                    in_=abls[jc][:, 1].rearrange("p a b -> p (a b)"))
            btT.append(t)

        # ---------------------------------------- Ax1T (1/H in Ww)
        ax1_ps = ps_b.tile([ATT, L], F32, tag="back")
        for jc in range(2):
            nc.tensor.matmul(ax1_ps, g_nat[:, jc, 0:ATT],
                             a1T[jc].rearrange("p a b -> p (a b)"),
                             start=(jc == 0), stop=(jc == 1))
        ax1_sb = sbig.tile([ATT, L], BF16, tag="ax1_sb")
        if last:
            nc.scalar.copy(out=ax1_sb, in_=ax1_ps)
        else:
            nc.vector.tensor_copy(out=ax1_sb, in_=ax1_ps)

        # ---------------------------------------- go2 (both layouts)
        go2T_ps = ps_b.tile([ATT, L], F32, tag="back")
        nc.tensor.matmul(go2T_ps, Ww, ax1_sb, start=True, stop=True)
        go2T = go2Ts[b % 2]
        if last:
            nc.scalar.activation(out=go2T[0:ATT, :], in_=go2T_ps,
                                 func=AF.Relu, bias=fs("Wb_col", 0, ATT))
        else:
            nc.vector.tensor_scalar(out=go2T[0:ATT, :], in0=go2T_ps,
                                    scalar1=fs("Wb_col", 0, ATT), scalar2=0.0,
                                    op0=OP.add, op1=OP.max)
        go2n = sbig.tile([128, L], BF16, tag="go2n")
        tpn = ps_ab.tile([128, L], BF16, tag="abT", name="tpn")
        for ic in range(2):
            nc.tensor.transpose(tpn[:, ic * 128:(ic + 1) * 128],
                                go2T[:, ic * 128:(ic + 1) * 128], ident)
        nc.vector.tensor_copy(out=go2n, in_=tpn)

        # ---------------------------------------- layer-2 rank-1 terms
        s2r_ps = ps_sm.tile([1, L], F32, tag="small", name="s2r_ps")
        nc.tensor.matmul(s2r_ps, w12s[:, 1:2], go2T[0:ATT, :], start=True,
                         stop=True)
        s2c_row = ssm.tile([1, L], BF16, tag="s2c_row")
        nc.vector.tensor_scalar_add(out=s2c_row, in0=s2r_ps, scalar1=cconst)
        s1c = []
        for jc in range(2):
            sc_ps = ps_sm.tile([128, 2], F32, tag="small", name="s1c_ps")
            nc.tensor.matmul(sc_ps, go2T[0:ATT, jc * 128:(jc + 1) * 128],
                             w12s, start=True, stop=True)
            t = ssm.tile([128, 1], BF16, tag=f"s1c{jc}", name=f"s1c{jc}")
            nc.vector.tensor_copy(out=t, in_=sc_ps[:, 0:1])
            s1c.append(t)
        tr_ps = ps_sm.tile([1, ATT], F32, tag="small", name="tr1_ps")
        for jc in range(2):
            nc.tensor.matmul(tr_ps, s1c[jc],
                             go2n[:, jc * 128:jc * 128 + ATT],
                             start=(jc == 0), stop=(jc == 1))
        cs_ps = ps_sm.tile([1, ATT], F32, tag="small", name="cs1_ps")
        for jc in range(2):
            nc.tensor.matmul(cs_ps, ones_col,
                             go2n[:, jc * 128:jc * 128 + ATT],
                             start=(jc == 0), stop=(jc == 1))
        tr_sb = ssm.tile([1, ATT], BF16, tag="tr_sb")
        nc.vector.tensor_copy(out=tr_sb, in_=tr_ps)
        cs_sb = ssm.tile([1, ATT], BF16, tag="cs_sb")
        nc.vector.tensor_copy(out=cs_sb, in_=cs_ps)

        # ---------------------------------------- Ax2T
        ax2_ps = ps_b.tile([ATT, L], F32, tag="back")
        for jc in range(2):
            nc.tensor.matmul(ax2_ps, go2n[:, jc * 128:jc * 128 + ATT],
                             btT[jc].rearrange("p a b -> p (a b)"),
                             start=(jc == 0), stop=False)
        nc.tensor.matmul(ax2_ps, tr_sb, ones_row, start=False, stop=False)
        nc.tensor.matmul(ax2_ps, cs_sb, s2c_row, start=False, stop=True)
        ax2_sb = sbig.tile([ATT, L], BF16, tag="ax2_sb")
        if last:
            nc.scalar.copy(out=ax2_sb, in_=ax2_ps)
        else:
            nc.vector.tensor_copy(out=ax2_sb, in_=ax2_ps)

        # ---------------------------------------- go3 + readout
        g3s = []
        for ic in range(2):
            g3_ps = ps_b.tile([128, ATT], F32, tag="back", name=f"g3_{ic}")
            nc.tensor.matmul(g3_ps, ax2_sb[:, ic * 128:(ic + 1) * 128],
                             Ww, start=True, stop=False)
            nc.tensor.matmul(g3_ps, ones_row[:, 0:128], Wb_row,
                             start=False, stop=True)
            g3 = sp.tile([128, ATT], BF16, tag="g3")
            if last:
                nc.scalar.activation(out=g3, in_=g3_ps, func=AF.Relu)
            else:
                nc.vector.tensor_scalar(out=g3, in0=g3_ps, scalar1=0.0,
                                        scalar2=0.0, op0=OP.max, op1=OP.add)
            g3s.append(g3)
        out1_ps = ps_sm.tile([ATT, 1], F32, tag="small", name="out1_ps")
        for ic in range(2):
            nc.tensor.matmul(out1_ps, g3s[ic],
                             bfs("am", 0, 128, 2 * b + ic, 2 * b + ic + 1),
                             start=(ic == 0), stop=(ic == 1))
        out1_sb = ssm.tile([ATT, 1], BF16, tag="out1_sb")
        nc.vector.tensor_copy(out=out1_sb, in_=out1_ps)
        clf_ps = ps_sm.tile([3, 1], F32, tag="small", name="clf_ps")
        nc.tensor.matmul(clf_ps, clf_w, out1_sb, start=True, stop=True)
        nc.vector.scalar_tensor_tensor(
            out=out_all[:, b:b + 1], in0=clf_ps, scalar=fs("rwn", 0, 3, b, b + 1),
            in1=fs("clf_b", 0, 3), op0=OP.mult, op1=OP.add)

    sts = [front(0), front(1)]
    back_scores(sts[0], 0)
    for b in range(bc):
        if b + 2 < bc:
            sts.append(front(b + 2))
        if b + 1 < bc:
            back_scores(sts[b + 1], b + 1)
        back_reduce(sts[b], b, last=(b == bc - 1))

    nc.gpsimd.dma_start(out=io["out"].ap().rearrange("b k -> k b"),
                        in_=out_all)

    for p in reversed(pools):
        p.release()


# ------------------------------------------------------------------- driver

_CACHE = {}


def build(cconst, was, bc=BC, num_devices=NCORES, debug=False):
    key = (round(cconst, 12), was, bc, num_devices)
    if key in _CACHE:
        return _CACHE[key]
    nc = bacc.Bacc("TRN2", target_bir_lowering=False, debug=debug,
                   num_devices=num_devices)
    io = {}
    for name, shape, dt in _IN_SPECS:
        io[name] = nc.dram_tensor(name, list(shape), dt, kind="ExternalInput")
    io["out"] = nc.dram_tensor("out", [bc, 3], F32, kind="ExternalOutput")
    with tile.TileContext(nc) as tc:
        _emit(tc, io, cconst, was, bc)
    nc.compile()
    _CACHE[key] = (nc, io)
    return nc, io


def run(inputs, **kwargs):
    per_core, cconst, was = _host_prep(inputs)
    nc, _ = build(cconst, was)
    res = run_bass_kernel_spmd(nc, per_core, core_ids=list(range(NCORES)),
                               **kwargs)
    return np.concatenate([r["out"] for r in res.results], axis=0), res


def kernel(**inputs):
    return run(inputs)[0]
